# revision 2
# baseline (speedup 1.0000x reference)
"""Trainium2 Bass kernel for nn_BLTModel (BLT encoder-decoder), 8 NeuronCores.

Core c -> batch b=c//2, token-half h=c%2. Patch-embed gathers + encoder +
decoder are data-parallel over batch (4 pairs) with a token split inside each
pair; per-pair bf16 AllGathers exchange activations. Decoder tokens are
assigned by STRIDE blocks (core h owns 128-blocks [h, h+2, h+4, h+6]) so the
causal structure lets both pair members skip the same 12/32 score chunks per
head. Key-padding masks are folded into the V rows (incl. the denominator
ones-column), so softmax exp needs no bias and runs one wide call per
(head, kv-chunk). The final projection is TOKEN-sharded: each core computes
logits for its own 512 tokens x full 32000 vocab with Wout streamed from HBM
under the matmul, written back as bf16 (host converts to f32) -- no final
8-rank AllGather. Weights double-buffer so layer l+1 loads during layer l.
All matmuls bf16 with f32 accumulation; activations kept transposed with
LayerNorm in natural layout.
"""
import sys

if '/opt/trn_rl_repo' not in sys.path:
    sys.path.insert(0, '/opt/trn_rl_repo')

import numpy as np
import ml_dtypes

import concourse.bass as bass
import concourse.mybir as mybir
import concourse.tile as tile
from concourse import bacc
from concourse.bass_utils import run_bass_kernel_spmd
from concourse.masks import make_identity, make_upper_triangular

FP32 = mybir.dt.float32
BF16 = mybir.dt.bfloat16
I32 = mybir.dt.int32
BF = ml_dtypes.bfloat16
AF = mybir.ActivationFunctionType
OP = mybir.AluOpType
AX = mybir.AxisListType

B, P, T, K = 4, 256, 1024, 8
D, FFD, NH = 512, 2048, 8
LE, LD = 2, 2
VOCAB, BUCKETS = 32000, 50000
HD = D // NH
N_CORES = 8
PP = P // 2
TOK = T // 2
NK = D // 128
NKF = FFD // 128
NEG = -1e9
EPS = 1e-5
PAIRS = [[2 * i, 2 * i + 1] for i in range(4)]

# stage E: stream Wout in chunks of WCH vocab cols, matmul tiles of VT cols
WCH = 2000
VT = 500
NCH = VOCAB // WCH          # 16
NVT = WCH // VT             # 4 tiles per chunk

_CACHE = {}


def build_program(bout_zero, masks_ones, debug=False):
    nc = bacc.Bacc("TRN2", target_bir_lowering=False, debug=False,
                   num_devices=N_CORES)

    # ---- inputs ----
    t_tables = nc.dram_tensor("tables_st", [3 * BUCKETS, D], BF16, kind="ExternalInput")
    t_ngids = nc.dram_tensor("ng_ids", [PP, 3 * K], I32, kind="ExternalInput")
    t_ppos = nc.dram_tensor("ppos_own", [PP, D], FP32, kind="ExternalInput")
    t_pmask = nc.dram_tensor("pmask_own", [PP, 1], FP32, kind="ExternalInput")
    t_pm01 = nc.dram_tensor("pm01", [128, 2], FP32, kind="ExternalInput")
    t_temb = nc.dram_tensor("token_emb", [VOCAB, D], BF16, kind="ExternalInput")
    t_tall = nc.dram_tensor("tids_all", [128, 8], I32, kind="ExternalInput")
    t_town = nc.dram_tensor("tids_own", [128, 4], I32, kind="ExternalInput")
    t_tposT = nc.dram_tensor("tposT", [D, T], BF16, kind="ExternalInput")
    t_tpos_own = nc.dram_tensor("tpos_own", [TOK, D], FP32, kind="ExternalInput")
    t_tm01 = nc.dram_tensor("tm01", [128, 8], FP32, kind="ExternalInput")
    t_half01 = nc.dram_tensor("half01", [128, 1], FP32, kind="ExternalInput")

    t_encW = nc.dram_tensor("enc_Wp", [LE, D, 4 * D], BF16, kind="ExternalInput")
    t_encW1 = nc.dram_tensor("enc_W1", [LE, D, FFD], BF16, kind="ExternalInput")
    t_encW2 = nc.dram_tensor("enc_W2", [LE, FFD, D], BF16, kind="ExternalInput")
    t_saW = nc.dram_tensor("dec_saWp", [LD, D, 4 * D], BF16, kind="ExternalInput")
    t_caW = nc.dram_tensor("dec_caWp", [LD, D, 4 * D], BF16, kind="ExternalInput")
    t_decW1 = nc.dram_tensor("dec_W1", [LD, D, FFD], BF16, kind="ExternalInput")
    t_decW2 = nc.dram_tensor("dec_W2", [LD, FFD, D], BF16, kind="ExternalInput")
    t_wout = nc.dram_tensor("wout", [D, VOCAB], BF16, kind="ExternalInput")

    t_qkvb = nc.dram_tensor("qkvb", [6, 3, NK, 128], FP32, kind="ExternalInput")
    t_ob = nc.dram_tensor("ob_bc", [6, 128, D], FP32, kind="ExternalInput")
    t_b1 = nc.dram_tensor("b1pp", [4, NKF, 128], FP32, kind="ExternalInput")
    t_b2 = nc.dram_tensor("b2_bc", [4, 128, D], FP32, kind="ExternalInput")
    t_lng = nc.dram_tensor("ln_g_bc", [10, 128, D], FP32, kind="ExternalInput")
    t_lnb = nc.dram_tensor("ln_b_bc", [10, 128, D], FP32, kind="ExternalInput")
    if not bout_zero:
        t_boutb = nc.dram_tensor("bout_bc", [128, VOCAB], FP32, kind="ExternalInput")

    t_out = nc.dram_tensor("logits", [4, 128, VOCAB], BF16, kind="ExternalOutput")
    dbg = {}
    if debug:
        dbg['pe'] = nc.dram_tensor("d_pe", [PP, D], FP32, kind="ExternalOutput")
        dbg['mem'] = nc.dram_tensor("d_mem", [PP, D], FP32, kind="ExternalOutput")
        dbg['y0'] = nc.dram_tensor("d_y0", [TOK, D], FP32, kind="ExternalOutput")
        dbg['dec1'] = nc.dram_tensor("d_dec1", [TOK, D], FP32, kind="ExternalOutput")

    with tile.TileContext(nc) as tc, \
         tc.tile_pool(name="const", bufs=1) as const, \
         tc.tile_pool(name="sml", bufs=3) as sml, \
         tc.tile_pool(name="mid", bufs=1) as mid, \
         tc.tile_pool(name="ps_lin", bufs=2, space="PSUM") as ps_lin, \
         tc.tile_pool(name="ps_sc", bufs=2, space="PSUM") as ps_sc, \
         tc.tile_pool(name="ps_z", bufs=2, space="PSUM") as ps_z, \
         tc.tile_pool(name="ps_misc", bufs=2, space="PSUM") as ps_misc, \
         tc.tile_pool(name="dram", bufs=1, space="DRAM") as dram:

        # ================= constants (small, whole-program) =================
        ident = const.tile([128, 128], FP32)
        make_identity(nc, ident[:])
        identb = const.tile([128, 128], BF16)
        make_identity(nc, identb[:])
        triu = const.tile([128, 128], BF16)
        make_upper_triangular(nc, triu[:], val=1.0, diag=True)
        ones1 = const.tile([1, HD], BF16)
        nc.vector.memset(ones1[:], 1.0)
        epsc = const.tile([128, 1], FP32)
        nc.vector.memset(epsc[:], EPS)
        pm01 = const.tile([128, 2], FP32)
        nc.sync.dma_start(pm01[:], t_pm01[:])
        tm01 = const.tile([128, 8], FP32)
        nc.sync.dma_start(tm01[:], t_tm01[:])
        half01 = const.tile([128, 1], FP32)
        nc.sync.dma_start(half01[:], t_half01[:])
        qkvb = const.tile([128, 6, 3, NK], FP32)
        nc.sync.dma_start(qkvb[:], t_qkvb[:].rearrange("a m g p -> p a m g"))
        b1c = const.tile([128, 4, NKF], FP32)
        nc.sync.dma_start(b1c[:], t_b1[:].rearrange("a g p -> p a g"))

        # ============ persistent activations (cross-phase) ============
        pe_nat = mid.tile([128, 1, D], FP32, tag="enc_nat_a")
        enc_nat_b = mid.tile([128, 1, D], FP32, tag="enc_nat_b")
        dec_x = mid.tile([128, 4, D], FP32, tag="dec_nat_a")
        dec_nat_b = mid.tile([128, 4, D], FP32, tag="dec_nat_b")
        dec_nat_c = mid.tile([128, 4, D], FP32, tag="dec_nat_c")
        y0T = mid.tile([128, NK, T], BF16, tag="y0T")
        memT_holder = []

        # ================= helpers =================
        def transpose_bf(xT_dst, src_bf, tg0, ngrp):
            """PE-transpose bf16 natural [128, g, D] -> xT_dst[:, kk, tok]."""
            with nc.allow_low_precision(reason="bf16 transpose, no accumulation"):
                for g in range(ngrp):
                    for kk in range(NK):
                        tp = ps_misc.tile([128, 128], BF16, tag="tp")
                        nc.tensor.transpose(
                            tp[:], src_bf[:, g, kk * 128:(kk + 1) * 128], identb[:])
                        nc.scalar.copy(
                            xT_dst[:, kk, (tg0 + g) * 128:(tg0 + g + 1) * 128], tp[:])

        def layer_norm(pool, sum_nat, bf_out, ngrp, ln_i):
            """LN in place on sum_nat (f32); also writes bf16 copy to bf_out."""
            lg_t = pool.tile([128, D], FP32, tag="lng")
            nc.sync.dma_start(lg_t[:], t_lng[ln_i])
            lb_t = pool.tile([128, D], FP32, tag="lnb")
            nc.sync.dma_start(lb_t[:], t_lnb[ln_i])
            for g in range(ngrp):
                x = sum_nat[:, g, :]
                s1 = sml.tile([128, 1], FP32, tag="ln_s1")
                nc.vector.reduce_sum(out=s1[:], in_=x, axis=AX.X)
                mean = sml.tile([128, 1], FP32, tag="ln_mean")
                nc.scalar.mul(mean[:], s1[:], 1.0 / D)
                xc = sml.tile([128, D], FP32, tag="ln_xc")
                nc.vector.tensor_scalar(out=xc[:], in0=x, scalar1=mean[:, :1],
                                        scalar2=None, op0=OP.subtract)
                sq = sml.tile([128, D], FP32, tag="ln_sq")
                ss = sml.tile([128, 1], FP32, tag="ln_ss")
                nc.scalar.activation(sq[:], xc[:], AF.Square, accum_out=ss[:, :1])
                sd = sml.tile([128, 1], FP32, tag="ln_sd")
                nc.scalar.activation(sd[:], ss[:], AF.Sqrt, bias=epsc[:, :1],
                                     scale=1.0 / D)
                rs = sml.tile([128, 1], FP32, tag="ln_rs")
                nc.vector.reciprocal(rs[:], sd[:])
                nc.vector.tensor_scalar(out=xc[:], in0=xc[:], scalar1=rs[:, :1],
                                        scalar2=None, op0=OP.mult)
                nc.vector.tensor_tensor(out=xc[:], in0=xc[:], in1=lg_t[:], op=OP.mult)
                nc.vector.tensor_tensor(out=x, in0=xc[:], in1=lb_t[:], op=OP.add)
                nc.vector.tensor_tensor(out=bf_out[:, g, :], in0=xc[:], in1=lb_t[:],
                                        op=OP.add)

        def attention(pool, a_i, xT_q, nq, xT_kv, nkv, w_sb, wo_sb, kmask01,
                      causal, resid_nat, out_sum):
            """kmask01: [128, nkc] 0/1 key mask folded into V rows.
            causal: True for decoder SA (stride-block chunk schedule)."""
            nkc = nkv // 128
            ntg = nq // 128
            qT = pool.tile([128, NK, nq], BF16, tag="qT")
            kT = pool.tile([128, NK, nkv], BF16, tag="kT")
            for m, dstT, src, ncols in ((0, qT, xT_q, nq), (1, kT, xT_kv, nkv)):
                for g in range(NK):
                    for c0 in range(0, ncols, 512):
                        cw = min(512, ncols - c0)
                        pp = ps_lin.tile([128, 512], FP32, tag="lin")
                        for kk in range(NK):
                            nc.tensor.matmul(
                                pp[:, :cw],
                                w_sb[:, kk, m * D + g * 128: m * D + (g + 1) * 128],
                                src[:, kk, c0:c0 + cw],
                                start=(kk == 0), stop=(kk == NK - 1))
                        nc.vector.tensor_scalar(
                            out=dstT[:, g, c0:c0 + cw], in0=pp[:, :cw],
                            scalar1=qkvb[:, a_i, m, g:g + 1], scalar2=None,
                            op0=OP.add)
            v_sb = pool.tile([128, nkc, NH, HD + 1], BF16, tag="v")
            nc.vector.memset(v_sb[:, :, :, HD:HD + 1], 1.0)
            for c in range(nkc):
                pp = ps_lin.tile([128, 512], FP32, tag="lin")
                for kk in range(NK):
                    nc.tensor.matmul(
                        pp[:], xT_kv[:, kk, c * 128:(c + 1) * 128],
                        w_sb[:, kk, 2 * D:3 * D],
                        start=(kk == 0), stop=(kk == NK - 1))
                nc.vector.tensor_copy(
                    out=v_sb[:, c, :, :HD],
                    in_=pp[:].rearrange("p (h d) -> p h d", h=NH))
                if not masks_ones:
                    nc.vector.tensor_scalar(
                        out=v_sb[:, c, :, :].rearrange("p h d -> p (h d)"),
                        in0=v_sb[:, c, :, :].rearrange("p h d -> p (h d)"),
                        scalar1=kmask01[:, c:c + 1], scalar2=None, op0=OP.mult)
            ob_t = pool.tile([128, D], FP32, tag="ob")
            nc.sync.dma_start(ob_t[:], t_ob[a_i])
            zT = pool.tile([64, NH, nq], BF16, tag="zT")
            for h in range(NH):
                pl = (h % 2) * 64
                gq = h // 2
                zp = ps_z.tile([128, 512], FP32, tag="z")
                for c in range(nkc):
                    # causal (decoder SA): q-range starts at block g_min(c)
                    gmin = (c if c < 4 else c - 4) if causal else 0
                    q0 = gmin * 128
                    w = nq - q0
                    sp = ps_sc.tile([128, 512], FP32, tag="sc")
                    nc.tensor.matmul(
                        sp[:, :w],
                        kT[pl:pl + 64, gq, c * 128:(c + 1) * 128],
                        qT[pl:pl + 64, gq, q0:nq],
                        start=True, stop=True)
                    a_sb = sml.tile([128, 512], BF16, tag="a")
                    nc.scalar.activation(a_sb[:, :w], sp[:, :w], AF.Exp,
                                         scale=float(1.0 / np.sqrt(HD)))
                    if causal:
                        # diagonal 128 cols of this chunk's range
                        if c < 4:
                            nc.vector.tensor_tensor(
                                out=a_sb[:, :128], in0=a_sb[:, :128],
                                in1=triu[:], op=OP.mult)
                        else:
                            nc.vector.tensor_scalar(
                                out=a_sb[:, :128], in0=a_sb[:, :128],
                                scalar1=half01[:, :1], scalar2=None, op0=OP.mult)
                    nc.tensor.matmul(
                        zp[:HD + 1, q0:nq], v_sb[:, c, h, :], a_sb[:, :w],
                        start=(c == 0), stop=(c == nkc - 1))
                rc = sml.tile([1, 512], BF16, tag="rc")
                with nc.allow_low_precision(reason="softmax denom recip, validated"):
                    nc.vector.reciprocal(rc[:, :nq], zp[HD:HD + 1, :nq])
                bp = ps_misc.tile([64, 512], FP32, tag="bc")
                nc.tensor.matmul(bp[:, :nq], ones1[:], rc[:, :nq],
                                 start=True, stop=True)
                bsb = sml.tile([64, 512], BF16, tag="bsb")
                nc.scalar.copy(bsb[:, :nq], bp[:, :nq])
                nc.vector.tensor_tensor(out=zT[:, h, :nq], in0=zp[:HD, :nq],
                                        in1=bsb[:, :nq], op=OP.mult)
            for g in range(ntg):
                op_ = ps_lin.tile([128, 512], FP32, tag="lin")
                for h in range(NH):
                    nc.tensor.matmul(op_[:], zT[:, h, g * 128:(g + 1) * 128],
                                     wo_sb[:, h, :],
                                     start=(h == 0), stop=(h == NH - 1))
                nc.vector.tensor_tensor(out=out_sum[:, g, :], in0=op_[:],
                                        in1=resid_nat[:, g, :], op=OP.add)
                nc.vector.tensor_tensor(out=out_sum[:, g, :], in0=out_sum[:, g, :],
                                        in1=ob_t[:], op=OP.add)

        def ffn(pool, ff_i, xT, ntok, w1_sb, w2_sb, resid_nat, out_sum):
            b2_t = pool.tile([128, D], FP32, tag="b2")
            nc.sync.dma_start(b2_t[:], t_b2[ff_i])
            th = min(ntok, 256)          # token-half width for hT
            for t0 in range(0, ntok, th):
                hT = pool.tile([128, NKF, th], BF16, tag="hT")
                for fg in range(NKF):
                    pp = ps_lin.tile([128, 512], FP32, tag="lin")
                    for kk in range(NK):
                        nc.tensor.matmul(pp[:, :th],
                                         w1_sb[:, kk, fg * 128:(fg + 1) * 128],
                                         xT[:, kk, t0:t0 + th],
                                         start=(kk == 0), stop=(kk == NK - 1))
                    nc.scalar.activation(hT[:, fg, :], pp[:, :th], AF.Relu,
                                         bias=b1c[:, ff_i, fg:fg + 1], scale=1.0)
                for g in range(th // 128):
                    gg = t0 // 128 + g
                    pp = ps_lin.tile([128, 512], FP32, tag="lin")
                    for fg in range(NKF):
                        nc.tensor.matmul(pp[:], hT[:, fg, g * 128:(g + 1) * 128],
                                         w2_sb[:, fg, :],
                                         start=(fg == 0), stop=(fg == NKF - 1))
                    nc.vector.tensor_tensor(out=out_sum[:, gg, :], in0=pp[:],
                                            in1=resid_nat[:, gg, :], op=OP.add)
                    nc.vector.tensor_tensor(out=out_sum[:, gg, :],
                                            in0=out_sum[:, gg, :],
                                            in1=b2_t[:], op=OP.add)

        def pair_ag(dstpool, src_sb, ncols, tag, dst_tag=None):
            bi = dram.tile([NK * 128, ncols], BF16, tag=tag + "_i")
            nc.gpsimd.dma_start(
                bi.opt().rearrange("(k p) t -> p k t", p=128), src_sb[:])
            bo = dram.tile([2 * NK * 128, ncols], BF16, tag=tag + "_o")
            nc.gpsimd.collective_compute(
                "AllGather", OP.bypass, replica_groups=PAIRS,
                ins=[bi.opt()], outs=[bo.opt()])
            dst = dstpool.tile([128, NK, 2 * ncols], BF16,
                               tag=(dst_tag or (tag + "_d")))
            for r in range(2):
                nc.sync.dma_start(
                    dst[:, :, r * ncols:(r + 1) * ncols],
                    bo.opt()[r * NK * 128:(r + 1) * NK * 128]
                    .rearrange("(k p) t -> p k t", p=128))
            return dst

        # ================= stage A: patch embedding =================
        xT_eown = mid.tile([128, NK, PP], BF16, tag="xT_eown")
        pe_bf = mid.tile([128, 1, D], BF16, tag="enc_bf")
        with tc.tile_pool(name="pa", bufs=1) as pa:
            ngid = pa.tile([PP, 3 * K], I32)
            nc.sync.dma_start(ngid[:], t_ngids[:])
            gth = pa.tile([128, 3 * K, D], BF16, tag="gth")
            for k in range(3 * K):
                nc.gpsimd.indirect_dma_start(
                    out=gth[:, k, :], out_offset=None, in_=t_tables[:],
                    in_offset=bass.IndirectOffsetOnAxis(ap=ngid[:, k:k + 1], axis=0))
            acc0 = pa.tile([128, D], FP32, tag="pacc0")
            acc1 = pa.tile([128, D], FP32, tag="pacc1")
            nc.vector.tensor_tensor(out=acc0[:], in0=gth[:, 0, :], in1=gth[:, 2, :], op=OP.add)
            nc.vector.tensor_tensor(out=acc1[:], in0=gth[:, 1, :], in1=gth[:, 3, :], op=OP.add)
            for k in range(4, 3 * K, 2):
                nc.vector.tensor_tensor(out=acc0[:], in0=acc0[:], in1=gth[:, k, :], op=OP.add)
            for k in range(5, 3 * K, 2):
                nc.vector.tensor_tensor(out=acc1[:], in0=acc1[:], in1=gth[:, k, :], op=OP.add)
            nc.vector.tensor_tensor(out=pe_nat[:, 0, :], in0=acc0[:], in1=acc1[:], op=OP.add)
            pmask = pa.tile([PP, 1], FP32)
            nc.sync.dma_start(pmask[:], t_pmask[:])
            nc.vector.tensor_scalar(out=pe_nat[:, 0, :], in0=pe_nat[:, 0, :],
                                    scalar1=pmask[:, :1], scalar2=None, op0=OP.mult)
            ppos = pa.tile([PP, D], FP32, tag="ppos")
            nc.sync.dma_start(ppos[:], t_ppos[:])
            nc.vector.tensor_tensor(out=pe_nat[:, 0, :], in0=pe_nat[:, 0, :],
                                    in1=ppos[:], op=OP.add)
            if debug:
                nc.sync.dma_start(dbg['pe'][:], pe_nat[:, 0, :])
            nc.vector.tensor_copy(out=pe_bf[:, 0, :], in_=pe_nat[:, 0, :])
            transpose_bf(xT_eown, pe_bf, 0, 1)

        # ================= stage C0: token embedding =================
        dec_bf = mid.tile([128, 4, D], BF16, tag="dec_bf")
        with tc.tile_pool(name="pc", bufs=1) as pc:
            tall = pc.tile([128, 8], I32)
            nc.sync.dma_start(tall[:], t_tall[:])
            town = pc.tile([128, 4], I32)
            nc.sync.dma_start(town[:], t_town[:])
            y0n = pc.tile([128, 8, D], BF16, tag="y0n")
            for c in range(8):
                nc.gpsimd.indirect_dma_start(
                    out=y0n[:, c, :], out_offset=None, in_=t_temb[:],
                    in_offset=bass.IndirectOffsetOnAxis(ap=tall[:, c:c + 1], axis=0))
            transpose_bf(y0T, y0n, 0, 8)
            tposT = pc.tile([128, NK, T], BF16, tag="tposT")
            nc.sync.dma_start(tposT[:], t_tposT[:].rearrange("(k p) t -> p k t", p=128))
            nc.vector.tensor_tensor(out=y0T[:].rearrange("p k t -> p (k t)"),
                                    in0=y0T[:].rearrange("p k t -> p (k t)"),
                                    in1=tposT[:].rearrange("p k t -> p (k t)"), op=OP.add)
            y0o = pc.tile([128, 4, D], BF16, tag="y0o")
            for c in range(4):
                nc.gpsimd.indirect_dma_start(
                    out=y0o[:, c, :], out_offset=None, in_=t_temb[:],
                    in_offset=bass.IndirectOffsetOnAxis(ap=town[:, c:c + 1], axis=0))
            tpos_o = pc.tile([128, 4, D], FP32, tag="tpos_o")
            nc.sync.dma_start(tpos_o[:], t_tpos_own[:].rearrange("(g p) n -> p g n", p=128))
            for g in range(4):
                nc.vector.tensor_tensor(out=dec_x[:, g, :], in0=tpos_o[:, g, :],
                                        in1=y0o[:, g, :], op=OP.add)
            if debug:
                nc.sync.dma_start(dbg['y0'][:].rearrange("(g p) n -> p g n", p=128),
                                  dec_x[:])

        # ================= stage B: encoder =================
        with tc.tile_pool(name="pe_", bufs=1) as pw, \
             tc.tile_pool(name="pew", bufs=2) as pww:
            xT_kv_enc = pair_ag(pw, xT_eown, PP, "ag0")
            for l in range(LE):
                w_sb = pww.tile([128, NK, 4 * D], BF16, tag="wqkv")
                nc.sync.dma_start(w_sb[:], t_encW[l].rearrange("(k p) n -> p k n", p=128))
                wo_sb = pww.tile([64, NH, D], BF16, tag="wo")
                nc.sync.dma_start(
                    wo_sb[:], t_encW[l, :, 3 * D:4 * D].rearrange("(h p) n -> p h n", p=64))
                w1_sb = pww.tile([128, NK, FFD], BF16, tag="w1")
                nc.sync.dma_start(w1_sb[:], t_encW1[l].rearrange("(k p) n -> p k n", p=128))
                w2_sb = pww.tile([128, NKF, D], BF16, tag="w2")
                nc.sync.dma_start(w2_sb[:], t_encW2[l].rearrange("(k p) n -> p k n", p=128))

                attention(pw, l, xT_eown, PP, xT_kv_enc, P, w_sb, wo_sb, pm01,
                          False, pe_nat, enc_nat_b)
                layer_norm(pw, enc_nat_b, pe_bf, 1, 2 * l)
                xT_mid_t = pw.tile([128, NK, PP], BF16, tag="xT_emid")
                transpose_bf(xT_mid_t, pe_bf, 0, 1)
                ffn(pw, l, xT_mid_t, PP, w1_sb, w2_sb, enc_nat_b, pe_nat)
                layer_norm(pw, pe_nat, pe_bf, 1, 2 * l + 1)
                xT_eown = mid.tile([128, NK, PP], BF16, tag=f"xT_eo{l}")
                transpose_bf(xT_eown, pe_bf, 0, 1)
                if l == 0:
                    xT_kv_enc = pair_ag(pw, xT_eown, PP, "ag1")
            memT = pair_ag(mid, xT_eown, PP, "ag2")
            memT_holder.append(memT)
            if debug:
                nc.sync.dma_start(dbg['mem'][:], pe_nat[:, 0, :])

        # ================= stage D: decoder =================
        memT = memT_holder[0]
        xT_down = mid.tile([128, NK, TOK], BF16, tag="xT_down")
        with tc.tile_pool(name="pd", bufs=1) as pw, \
             tc.tile_pool(name="pdw", bufs=2) as pww:
            transpose_bf(xT_down, dec_bf, 0, 4)  # dec_bf written below first
            kv_dec = y0T
            for g in range(4):
                nc.vector.tensor_copy(out=dec_bf[:, g, :], in_=dec_x[:, g, :])
            for l in range(LD):
                w_sb = pww.tile([128, NK, 4 * D], BF16, tag="wqkv")
                nc.sync.dma_start(w_sb[:], t_saW[l].rearrange("(k p) n -> p k n", p=128))
                wo_sb = pww.tile([64, NH, D], BF16, tag="wo")
                nc.sync.dma_start(
                    wo_sb[:], t_saW[l, :, 3 * D:4 * D].rearrange("(h p) n -> p h n", p=64))
                cw_sb = pww.tile([128, NK, 4 * D], BF16, tag="wca")
                nc.sync.dma_start(cw_sb[:], t_caW[l].rearrange("(k p) n -> p k n", p=128))
                cwo_sb = pww.tile([64, NH, D], BF16, tag="wo2")
                nc.sync.dma_start(
                    cwo_sb[:], t_caW[l, :, 3 * D:4 * D].rearrange("(h p) n -> p h n", p=64))
                w1_sb = pww.tile([128, NK, FFD], BF16, tag="w1")
                nc.sync.dma_start(w1_sb[:], t_decW1[l].rearrange("(k p) n -> p k n", p=128))
                w2_sb = pww.tile([128, NKF, D], BF16, tag="w2")
                nc.sync.dma_start(w2_sb[:], t_decW2[l].rearrange("(k p) n -> p k n", p=128))

                attention(pw, 2 + l, xT_down, TOK, kv_dec, T, w_sb, wo_sb, tm01,
                          True, dec_x, dec_nat_b)
                layer_norm(pw, dec_nat_b, dec_bf, 4, 4 + 3 * l)
                xT_sa = pw.tile([128, NK, TOK], BF16, tag="xT_dmid")
                transpose_bf(xT_sa, dec_bf, 0, 4)
                attention(pw, 4 + l, xT_sa, TOK, memT, P, cw_sb, cwo_sb, pm01,
                          False, dec_nat_b, dec_nat_c)
                layer_norm(pw, dec_nat_c, dec_bf, 4, 4 + 3 * l + 1)
                xT_ca = pw.tile([128, NK, TOK], BF16, tag="xT_dmid")
                transpose_bf(xT_ca, dec_bf, 0, 4)
                ffn(pw, 2 + l, xT_ca, TOK, w1_sb, w2_sb, dec_nat_c, dec_x)
                layer_norm(pw, dec_x, dec_bf, 4, 4 + 3 * l + 2)
                xT_down = mid.tile([128, NK, TOK], BF16, tag=f"xT_dn{l}")
                transpose_bf(xT_down, dec_bf, 0, 4)
                if l == 0:
                    kv_dec = pair_ag(mid, xT_down, TOK, "ag3", dst_tag="y0T")
            if debug:
                nc.sync.dma_start(
                    dbg['dec1'][:].rearrange("(g p) n -> p g n", p=128), dec_x[:])

        # ================= stage E: final projection (token-sharded) ========
        with tc.tile_pool(name="pf", bufs=3) as pw, \
             tc.tile_pool(name="pfo", bufs=3) as pfo, \
             tc.tile_pool(name="ps_e", bufs=6, space="PSUM") as ps_e:
            for ci in range(NCH):
                wsb = pw.tile([128, NK, WCH], BF16, tag="wout")
                nc.sync.dma_start(
                    wsb[:], t_wout[:, ci * WCH:(ci + 1) * WCH]
                    .rearrange("(k p) n -> p k n", p=128))
                if not bout_zero:
                    bsb = pw.tile([128, WCH], FP32, tag="bout")
                    nc.sync.dma_start(bsb[:], t_boutb[:, ci * WCH:(ci + 1) * WCH])
                for tg in range(4):
                    stg = pfo.tile([128, WCH], BF16, tag="stg")
                    for j in range(NVT):
                        pp = ps_e.tile([128, VT], FP32, tag="pe")
                        for kk in range(NK):
                            nc.tensor.matmul(
                                pp[:],
                                xT_down[:, kk, tg * 128:(tg + 1) * 128],
                                wsb[:, kk, j * VT:(j + 1) * VT],
                                start=(kk == 0), stop=(kk == NK - 1))
                        if bout_zero:
                            if j % 2 == 0:
                                nc.vector.tensor_copy(
                                    out=stg[:, j * VT:(j + 1) * VT], in_=pp[:])
                            else:
                                nc.scalar.copy(stg[:, j * VT:(j + 1) * VT], pp[:])
                        else:
                            nc.vector.tensor_tensor(
                                out=stg[:, j * VT:(j + 1) * VT], in0=pp[:],
                                in1=bsb[:, j * VT:(j + 1) * VT], op=OP.add)
                    nc.sync.dma_start(
                        t_out[tg, :, ci * WCH:(ci + 1) * WCH], stg[:])

    nc.compile()
    return nc


# ---------------------------------------------------------------------------
# host side
# ---------------------------------------------------------------------------

def _bf(x):
    return np.ascontiguousarray(np.asarray(x, np.float32)).astype(BF)


def _f32(x):
    return np.ascontiguousarray(np.asarray(x, np.float32))


def _prep_inputs(inputs):
    ngram_ids = np.asarray(inputs['ngram_ids'])
    patch_mask = np.asarray(inputs['patch_mask'])
    target_ids = np.asarray(inputs['target_ids'])
    target_mask = np.asarray(inputs['target_mask'])
    tables = _f32(inputs['tables'])
    patch_pos = _f32(inputs['patch_pos'])
    token_emb = _f32(inputs['token_emb'])
    token_pos = _f32(inputs['token_pos'])
    enc_W = _f32(inputs['enc_W']); enc_b = _f32(inputs['enc_b'])
    enc_W1 = _f32(inputs['enc_W1']); enc_b1 = _f32(inputs['enc_b1'])
    enc_W2 = _f32(inputs['enc_W2']); enc_b2 = _f32(inputs['enc_b2'])
    enc_lng = _f32(inputs['enc_lng']); enc_lnb = _f32(inputs['enc_lnb'])
    dec_saW = _f32(inputs['dec_saW']); dec_sab = _f32(inputs['dec_sab'])
    dec_caW = _f32(inputs['dec_caW']); dec_cab = _f32(inputs['dec_cab'])
    dec_W1 = _f32(inputs['dec_W1']); dec_b1 = _f32(inputs['dec_b1'])
    dec_W2 = _f32(inputs['dec_W2']); dec_b2 = _f32(inputs['dec_b2'])
    dec_lng = _f32(inputs['dec_lng']); dec_lnb = _f32(inputs['dec_lnb'])
    Wout = _f32(inputs['Wout']); bout = _f32(inputs['bout'])

    stacked = _bf(tables.reshape(3 * BUCKETS, D))
    temb_bf = _bf(token_emb)
    wout_bf = _bf(Wout)

    encWp = np.stack([_bf(np.concatenate([enc_W[l, i] for i in range(4)], axis=1))
                      for l in range(LE)])
    saWp = np.stack([_bf(np.concatenate([dec_saW[l, i] for i in range(4)], axis=1))
                     for l in range(LD)])
    caWp = np.stack([_bf(np.concatenate([dec_caW[l, i] for i in range(4)], axis=1))
                     for l in range(LD)])
    encW1b = _bf(enc_W1); encW2b = _bf(enc_W2)
    decW1b = _bf(dec_W1); decW2b = _bf(dec_W2)

    inst_Wb = [(enc_W[0], enc_b[0]), (enc_W[1], enc_b[1]),
               (dec_saW[0], dec_sab[0]), (dec_saW[1], dec_sab[1]),
               (dec_caW[0], dec_cab[0]), (dec_caW[1], dec_cab[1])]
    qkvb = np.zeros((6, 3, NK, 128), np.float32)
    ob_bc = np.zeros((6, 128, D), np.float32)
    for a, (W4, b4) in enumerate(inst_Wb):
        qkvb[a] = b4[0:3].reshape(3, NK, 128)
        ob_eff = b4[3] + b4[2] @ W4[3]
        ob_bc[a] = np.broadcast_to(ob_eff[None, :], (128, D))
    b1pp = np.stack([enc_b1[0], enc_b1[1], dec_b1[0], dec_b1[1]]).reshape(4, NKF, 128)
    b2_bc = np.stack([np.broadcast_to(v[None, :], (128, D)) for v in
                      [enc_b2[0], enc_b2[1], dec_b2[0], dec_b2[1]]])
    ln_list = [enc_lng[0, 0], enc_lng[0, 1], enc_lng[1, 0], enc_lng[1, 1],
               dec_lng[0, 0], dec_lng[0, 1], dec_lng[0, 2],
               dec_lng[1, 0], dec_lng[1, 1], dec_lng[1, 2]]
    lnb_list = [enc_lnb[0, 0], enc_lnb[0, 1], enc_lnb[1, 0], enc_lnb[1, 1],
                dec_lnb[0, 0], dec_lnb[0, 1], dec_lnb[0, 2],
                dec_lnb[1, 0], dec_lnb[1, 1], dec_lnb[1, 2]]
    ln_g_bc = np.stack([np.broadcast_to(v[None, :], (128, D)) for v in ln_list])
    ln_b_bc = np.stack([np.broadcast_to(v[None, :], (128, D)) for v in lnb_list])

    bout_zero = bool(np.all(bout == 0.0))
    masks_ones = bool(np.all(patch_mask) and np.all(target_mask))

    in_maps = []
    for c in range(N_CORES):
        b = c // 2
        h = c % 2
        qblocks = [h, h + 2, h + 4, h + 6]
        pblocks = [1 - h, 3 - h + 2, 5 - h + 2 * 2, 7 - h + 2 * 3]
        pblocks = [(1 - h) + 2 * i for i in range(4)]
        kvblocks = qblocks + pblocks
        qidx = np.concatenate([np.arange(blk * 128, (blk + 1) * 128)
                               for blk in qblocks])
        kvidx = np.concatenate([np.arange(blk * 128, (blk + 1) * 128)
                                for blk in kvblocks])
        ng = ngram_ids[b, h * PP:(h + 1) * PP].astype(np.int64)
        ng = ng + (np.arange(3) * BUCKETS)[None, :, None]
        ng = np.ascontiguousarray(ng.reshape(PP, 3 * K)).astype(np.int32)
        pm_own = patch_mask[b, h * PP:(h + 1) * PP].astype(np.float32)[:, None]
        # encoder/CA kv order = [own half, partner half]
        pm01 = np.stack([patch_mask[b, h * PP:(h + 1) * PP],
                         patch_mask[b, (1 - h) * PP:(2 - h) * PP]], axis=1)
        pm01 = np.ascontiguousarray(pm01.astype(np.float32))
        tm01 = np.ascontiguousarray(
            target_mask[b, kvidx].astype(np.float32).reshape(8, 128).T)
        tids_all = np.ascontiguousarray(
            target_ids[b, kvidx].reshape(8, 128).T).astype(np.int32)
        tids_own = np.ascontiguousarray(
            target_ids[b, qidx].reshape(4, 128).T).astype(np.int32)
        tposT_bf = _bf(token_pos[kvidx].T)
        half01 = np.full((128, 1), float(h), np.float32)
        in_map = {
            "tables_st": stacked,
            "ng_ids": ng,
            "ppos_own": np.ascontiguousarray(patch_pos[h * PP:(h + 1) * PP]),
            "pmask_own": np.ascontiguousarray(pm_own),
            "pm01": pm01,
            "token_emb": temb_bf,
            "tids_all": tids_all,
            "tids_own": tids_own,
            "tposT": tposT_bf,
            "tpos_own": np.ascontiguousarray(token_pos[qidx]),
            "tm01": tm01,
            "half01": half01,
            "enc_Wp": encWp,
            "enc_W1": encW1b,
            "enc_W2": encW2b,
            "dec_saWp": saWp,
            "dec_caWp": caWp,
            "dec_W1": decW1b,
            "dec_W2": decW2b,
            "wout": wout_bf,
            "qkvb": qkvb,
            "ob_bc": ob_bc,
            "b1pp": b1pp,
            "b2_bc": b2_bc,
            "ln_g_bc": ln_g_bc,
            "ln_b_bc": ln_b_bc,
        }
        if not bout_zero:
            in_map["bout_bc"] = np.ascontiguousarray(
                np.broadcast_to(bout[None, :], (128, VOCAB))).astype(np.float32)
        in_maps.append(in_map)
    return in_maps, bout_zero, masks_ones


def run(inputs, debug=False, trace=False):
    in_maps, bout_zero, masks_ones = _prep_inputs(inputs)
    key = ("dbg" if debug else "rel", bout_zero, masks_ones)
    if key not in _CACHE:
        _CACHE[key] = build_program(bout_zero, masks_ones, debug=debug)
    nc = _CACHE[key]
    res = run_bass_kernel_spmd(nc, in_maps, core_ids=list(range(N_CORES)),
                               trace=trace)
    return res


def assemble(res):
    out = np.zeros((B, T, VOCAB), np.float32)
    for c in range(N_CORES):
        b = c // 2
        h = c % 2
        lg = res.results[c]["logits"].astype(np.float32)   # [4, 128, VOCAB]
        for g in range(4):
            blk = 2 * g + h
            out[b, blk * 128:(blk + 1) * 128, :] = lg[g]
    return out


def kernel(**inputs):
    return assemble(run(inputs))
